# revision 5
# baseline (speedup 1.0000x reference)
"""Trainium2 Bass kernel for nn_CRAMForCausalLM.

Sharding: 8-way data-parallel over tokens (each core owns 256 contiguous
tokens of one batch element, plus a 32-token halo so the EMA retention scan
is computed locally — contributions older than 32 steps are damped by 0.5^32
< 3e-10, below f32 noise).  The LM head is vocab-sharded (each core computes
4000 logit rows for all 2048 tokens) fed by a single AllGather of the final
hidden states.  Activations live feature-major ([H, tokens]) on chip so every
GEMM chains without transposes; LayerNorm stats use ones-matmuls on the
TensorEngine; the EMA recurrence uses the VectorEngine tensor_tensor_scan.
"""

import numpy as np

import concourse.bass as bass
import concourse.bacc as bacc
import concourse.tile as tile
import concourse.mybir as mybir
import concourse.bass_utils as bass_utils
import os as _os

LAST_EXEC_NS = None


def _maybe_install_trace_hook():
    import contextlib, ctypes, sys, types
    if "antenv.axon_hooks" in sys.modules:
        return
    lib = ctypes.CDLL("/opt/axon/libaxon_pjrt.so")
    if not hasattr(lib, "axon_start_nrt_profile"):
        return
    lib.axon_start_nrt_profile.argtypes = [ctypes.POINTER(ctypes.c_int64), ctypes.c_size_t]
    lib.axon_start_nrt_profile.restype = ctypes.c_int64
    lib.axon_stop_nrt_profile.argtypes = [ctypes.c_char_p]
    lib.axon_stop_nrt_profile.restype = ctypes.c_int64

    @contextlib.contextmanager
    def _hook(output_dir, device_ids):
        import jax
        jax.devices()
        if device_ids:
            ids = (ctypes.c_int64 * len(device_ids))(*device_ids)
            rc = lib.axon_start_nrt_profile(ids, len(device_ids))
        else:
            rc = lib.axon_start_nrt_profile(None, 0)
        if rc != 0:
            raise RuntimeError(f"axon_start_nrt_profile rc={rc}")
        try:
            yield
        finally:
            lib.axon_stop_nrt_profile(str(output_dir).encode())

    mod = types.ModuleType("antenv.axon_hooks")
    mod.get_axon_ntff_profile_hook = lambda: _hook
    mod.set_axon_ntff_profile_hook = lambda h: None
    sys.modules["antenv.axon_hooks"] = mod

AF = mybir.ActivationFunctionType
OP = mybir.AluOpType

B, S, H, F, L, V = 2, 1024, 1024, 4096, 8, 32000
EPS = 1e-5
NCORES = 8
HALO = 32
TM = 256            # main tokens per core
T = TM + HALO       # 288 tokens processed per core
TPAD = 384          # padded to 3 x 128 for the embedding gather
KH = H // 128       # 8 k-chunks over H
MH = H // 128       # 8 m-tiles over H
MF = F // 128       # 32 m-tiles over F
VS = V // NCORES    # 4000 vocab rows per core
VSP = 4096          # padded vocab rows per core
TALL = B * S        # 2048 total tokens

f32 = mybir.dt.float32
f32r = mybir.dt.float32r
bf16 = mybir.dt.bfloat16
i32 = mybir.dt.int32

_compiled = {}


def _swz(w, kp=128, mf=128):
    """[K, M] -> [mt, kp, kc*mf] so lhsT tile (mt, kc) = sbuf[:, kc*mf:(kc+1)*mf]."""
    K, M = w.shape
    kc, mt = K // kp, M // mf
    return np.ascontiguousarray(
        w.reshape(kc, kp, mt, mf).transpose(2, 1, 0, 3).reshape(mt, kp, kc * mf)
    )


def _cols(v, mt, width=128):
    """[M] -> [width, mt] so column j is v[j*width:(j+1)*width]."""
    return np.ascontiguousarray(v.reshape(mt, width).T)


def _build(ffn_bf16, ln_scaled):
    """Build + compile the Bass program.

    ffn_bf16: FFN weights/activations in bf16 (else float32r)
    ln_scaled: apply LayerNorm scale/bias tensors (else they are known
               to be identity and are skipped)
    """
    nc = bacc.Bacc("TRN2", target_bir_lowering=False, debug=False,
                   num_devices=NCORES)
    wdt = bf16 if ffn_bf16 else f32r

    # ---- DRAM I/O ----
    ids_d = nc.dram_tensor("ids", [3, 128], i32, kind="ExternalInput")
    pos_d = nc.dram_tensor("pos", [3, 128, H], f32, kind="ExternalInput")
    wemb_d = nc.dram_tensor("wemb", [V, H], f32, kind="ExternalInput")
    retw_d = nc.dram_tensor("retw", [L, MH, 128, KH * 128], f32r, kind="ExternalInput")
    retb_d = nc.dram_tensor("retb", [L, 128, MH], f32, kind="ExternalInput")
    w1_d = nc.dram_tensor("w1", [L, MF, 128, KH * 128], wdt, kind="ExternalInput")
    b1_d = nc.dram_tensor("b1", [L, 128, MF], f32, kind="ExternalInput")
    w2_d = nc.dram_tensor("w2", [L, MH, 128, MF * 128], wdt, kind="ExternalInput")
    b2_d = nc.dram_tensor("b2", [L, 128, MH], f32, kind="ExternalInput")
    lmw_d = nc.dram_tensor("lmw", [VSP // 128, 128, KH * 128], f32r, kind="ExternalInput")
    mask_d = nc.dram_tensor("mask", [128, 1], f32, kind="ExternalInput")
    if ln_scaled:
        # 2 (s, b) x (emb, per-layer ln1, per-layer ln2, fin): [128, MH] each
        lns_d = nc.dram_tensor("lns", [2 * L + 2, 2, 128, MH], f32, kind="ExternalInput")
    out_d = nc.dram_tensor("logits", [VSP, TALL], f32, kind="ExternalOutput")

    with tile.TileContext(nc) as tc:
        with tc.tile_pool(name="per", bufs=1) as per, \
             tc.tile_pool(name="gpool", bufs=1) as gpool:
            # persistent tiles
            xt = [per.tile([128, T], f32r, tag=f"xt{k}", name=f"xt{k}") for k in range(KH)]
            y1 = [per.tile([128, T], f32r, tag=f"y1{k}", name=f"y1{k}") for k in range(KH)]
            hres = [per.tile([128, T], f32r, tag=f"h{k}", name=f"h{k}") for k in range(KH)]
            g = [gpool.tile([128, T], wdt, tag=f"g{k}", name=f"g{k}") for k in range(MF)]
            half = per.tile([128, T], f32)
            nc.gpsimd.memset(half[:], 0.5)
            ones_f = per.tile([128, 1], f32)
            nc.gpsimd.memset(ones_f[:], 1.0)
            ones = per.tile([128, 1], f32r)
            nc.vector.tensor_copy(ones[:], ones_f[:])
            onesr_f = per.tile([1, 128], f32)
            nc.gpsimd.memset(onesr_f[:], 1.0)
            onesr = per.tile([1, 128], f32r)
            nc.vector.tensor_copy(onesr[:], onesr_f[:])
            mask = per.tile([128, 1], f32)
            nc.sync.dma_start(mask[:], mask_d.ap())
            epsc = per.tile([1, 1], f32)
            nc.gpsimd.memset(epsc[:], EPS)
            ident = per.tile([128, 128], f32)
            from concourse.masks import make_identity
            make_identity(nc, ident[:])
            if ln_scaled:
                lnt = per.tile([128, (2 * L + 2) * 2 * MH], f32)
                nc.sync.dma_start(
                    lnt[:],
                    lns_d.ap().rearrange("a b p m -> p (a b m)"))
            else:
                lnt = None

            def ln_cols(slot):
                # returns (scale_cols, bias_cols): APs [128, MH] or None
                if lnt is None:
                    return None, None
                off = slot * 2 * MH
                return lnt[:, off:off + MH], lnt[:, off + MH:off + 2 * MH]

            # ---------- LayerNorm helper (feature-major) ----------
            def layer_norm(ps_stat, ps_bc, tmp, yin, yout, slot):
                """yin: 8 tiles [128, T] f32r; yout: 8 tiles (any dtype).
                Normalizes over the partition(H) axis per token column."""
                scol, bcol = ln_cols(slot)
                sq = []
                for k in range(KH):
                    s = tmp.tile([128, T], f32r, tag="sq", name="sq")
                    nc.scalar.activation(s[:], yin[k][:].bitcast(f32), AF.Square)
                    sq.append(s)
                p_sy = ps_stat.tile([1, T], f32, tag="psy")
                p_sq = ps_stat.tile([1, T], f32, tag="psq")
                for k in range(KH):
                    nc.tensor.matmul(p_sy[:], ones[:], yin[k][:],
                                     start=(k == 0), stop=(k == KH - 1))
                for k in range(KH):
                    nc.tensor.matmul(p_sq[:], ones[:], sq[k][:],
                                     start=(k == 0), stop=(k == KH - 1))
                nm = tmp.tile([1, T], f32, tag="nm")
                nc.vector.tensor_scalar_mul(nm[:], p_sy[:], -1.0 / H)
                v1 = tmp.tile([1, T], f32, tag="v1")
                nc.vector.tensor_scalar_mul(v1[:], p_sq[:], 1.0 / H)
                m2 = tmp.tile([1, T], f32, tag="m2")
                nc.vector.tensor_tensor(m2[:], nm[:], nm[:], OP.mult)
                var = tmp.tile([1, T], f32, tag="var")
                nc.vector.tensor_tensor(var[:], v1[:], m2[:], OP.subtract)
                sd = tmp.tile([1, T], f32, tag="sd")
                nc.scalar.activation(sd[:], var[:], AF.Sqrt, bias=epsc[:])
                r = tmp.tile([1, T], f32, tag="r")
                nc.vector.reciprocal(r[:], sd[:])
                r_r = tmp.tile([1, T], f32r, tag="rr")
                nc.vector.tensor_copy(r_r[:], r[:])
                nm_r = tmp.tile([1, T], f32r, tag="nmr")
                nc.vector.tensor_copy(nm_r[:], nm[:])
                p_rb = ps_bc.tile([128, T], f32, tag="prb")
                nc.tensor.matmul(p_rb[:], onesr[:], r_r[:], start=True, stop=True)
                p_nmb = ps_bc.tile([128, T], f32, tag="pnmb")
                nc.tensor.matmul(p_nmb[:], onesr[:], nm_r[:], start=True, stop=True)
                for k in range(KH):
                    z = tmp.tile([128, T], f32, tag="z")
                    nc.vector.tensor_tensor(z[:], yin[k][:].bitcast(f32),
                                            p_nmb[:], OP.add)
                    if scol is None:
                        nc.vector.tensor_tensor(yout[k][:], z[:], p_rb[:], OP.mult)
                    else:
                        z2 = tmp.tile([128, T], f32, tag="z2")
                        nc.vector.tensor_tensor(z2[:], z[:], p_rb[:], OP.mult)
                        nc.vector.tensor_scalar(
                            yout[k][:], z2[:],
                            scol[:, k:k + 1], bcol[:, k:k + 1], OP.mult, OP.add)

            # ================= Embedding =================
            with tc.tile_pool(name="emb", bufs=2) as ep, \
                 tc.tile_pool(name="pse", bufs=3, space="PSUM") as pse:
                for c in range(3):
                    idx = ep.tile([128, 1], i32, tag="idx")
                    nc.sync.dma_start(idx[:], ids_d.ap()[c][:, None])
                    gt = ep.tile([128, H], f32, tag="gt")
                    nc.gpsimd.indirect_dma_start(
                        out=gt[:], out_offset=None, in_=wemb_d.ap(),
                        in_offset=bass.IndirectOffsetOnAxis(ap=idx[:, :1], axis=0))
                    pt = ep.tile([128, H], f32, tag="pt")
                    nc.sync.dma_start(pt[:], pos_d.ap()[c])
                    nc.vector.tensor_tensor(gt[:], gt[:], pt[:], OP.add)
                    cnt = T - 256 if c == 2 else 128
                    for k in range(KH):
                        ptr = pse.tile([128, 128], f32, tag="ptr")
                        nc.tensor.transpose(ptr[:], gt[:, k * 128:(k + 1) * 128],
                                            ident[:])
                        nc.vector.tensor_copy(
                            y1[k][:, c * 128:c * 128 + cnt], ptr[:, :cnt])

            # ================= Layers =================
            with tc.tile_pool(name="wret", bufs=2) as wret, \
                 tc.tile_pool(name="w1p", bufs=3) as w1p, \
                 tc.tile_pool(name="w2p", bufs=2) as w2p, \
                 tc.tile_pool(name="bias", bufs=2) as biasp, \
                 tc.tile_pool(name="tmp", bufs=3) as tmp, \
                 tc.tile_pool(name="psmm", bufs=3, space="PSUM") as psmm, \
                 tc.tile_pool(name="psst", bufs=1, space="PSUM") as ps_stat, \
                 tc.tile_pool(name="psbc", bufs=1, space="PSUM") as ps_bc:

                # embedding LN:  y1 -> xt
                layer_norm(ps_stat, ps_bc, tmp, y1, xt, 0)

                for l in range(L):
                    retb = biasp.tile([128, MH], f32, tag="retb")
                    nc.sync.dma_start(retb[:], retb_d.ap()[l])
                    b1 = biasp.tile([128, MF], f32, tag="b1")
                    nc.sync.dma_start(b1[:], b1_d.ap()[l])
                    b2 = biasp.tile([128, MH], f32, tag="b2")
                    nc.sync.dma_start(b2[:], b2_d.ap()[l])

                    # --- retention GEMM + sigmoid + EMA scan ---
                    st = []
                    for mt in range(MH):
                        wt = wret.tile([128, KH * 128], f32r, tag="wret")
                        nc.sync.dma_start(wt[:], retw_d.ap()[l, mt])
                        ps = psmm.tile([128, T], f32, tag="mm")
                        for kc in range(KH):
                            nc.tensor.matmul(
                                ps[:], wt[:, kc * 128:(kc + 1) * 128], xt[kc][:],
                                start=(kc == 0), stop=(kc == KH - 1))
                        s = tmp.tile([128, T], f32, tag="sig")
                        nc.scalar.activation(s[:], ps[:], AF.Sigmoid,
                                             bias=retb[:, mt:mt + 1])
                        nc.vector.tensor_scalar_mul(
                            s[:, :HALO], s[:, :HALO], mask[:, :1])
                        stt = tmp.tile([128, T], f32, tag="scan")
                        nc.vector.tensor_tensor_scan(
                            stt[:], half[:], s[:], 0.0, OP.mult, OP.add)
                        st.append(stt)
                        # y1 = x + 0.5*scan_state   (f32r rounded on write)
                        nc.vector.scalar_tensor_tensor(
                            y1[mt][:], stt[:], 0.5, xt[mt][:].bitcast(f32),
                            OP.mult, OP.add)

                    # --- LN1: y1 -> hres (f32) ---
                    layer_norm(ps_stat, ps_bc, tmp, y1, hres, 1 + 2 * l)

                    # h in GEMM dtype
                    if ffn_bf16:
                        hg = []
                        for k in range(KH):
                            hb = tmp.tile([128, T], bf16, tag="hb")
                            nc.vector.tensor_copy(hb[:], hres[k][:].bitcast(f32))
                            hg.append(hb)
                    else:
                        hg = hres

                    # --- FFN1 + gelu ---
                    for mt in range(MF):
                        wt = w1p.tile([128, KH * 128], wdt, tag="w1")
                        nc.sync.dma_start(wt[:], w1_d.ap()[l, mt])
                        ps = psmm.tile([128, T], f32, tag="mm")
                        for kc in range(KH):
                            nc.tensor.matmul(
                                ps[:], wt[:, kc * 128:(kc + 1) * 128], hg[kc][:],
                                start=(kc == 0), stop=(kc == KH - 1))
                        nc.scalar.activation(g[mt][:], ps[:], AF.Gelu_apprx_tanh,
                                             bias=b1[:, mt:mt + 1])

                    # --- FFN2 ---
                    for mt in range(MH):
                        wt = w2p.tile([128, MF * 128], wdt, tag="w2")
                        nc.sync.dma_start(wt[:], w2_d.ap()[l, mt])
                        ps = psmm.tile([128, T], f32, tag="mm")
                        for kc in range(MF):
                            nc.tensor.matmul(
                                ps[:], wt[:, kc * 128:(kc + 1) * 128], g[kc][:],
                                start=(kc == 0), stop=(kc == MF - 1))
                        # y1 = (ffn + b2) + h    (becomes LN2 input)
                        nc.vector.scalar_tensor_tensor(
                            y1[mt][:], ps[:], b2[:, mt:mt + 1],
                            hres[mt][:].bitcast(f32), OP.add, OP.add)

                    # --- LN2: y1 -> xt ---
                    layer_norm(ps_stat, ps_bc, tmp, y1, xt, 2 + 2 * l)

                # final LN: xt -> y1 (reuse y1 tiles as x_final, f32r)
                layer_norm(ps_stat, ps_bc, tmp, xt, y1, 2 * L + 1)

            # ================= AllGather of final hidden =================
            with tc.tile_pool(name="dram", bufs=1, space="DRAM") as dramp:
                bnc = dramp.tile([H, TM], f32r)
                for k in range(KH):
                    nc.sync.dma_start(bnc[k * 128:(k + 1) * 128, :],
                                      y1[k][:, HALO:T])
                xg = dramp.tile([NCORES, H, TM], f32r, addr_space="Shared")
                nc.gpsimd.collective_compute(
                    "AllGather", OP.bypass,
                    replica_groups=[list(range(NCORES))],
                    ins=[bnc.opt()], outs=[xg.opt()])

                # ================= LM head =================
                with tc.tile_pool(name="lmx", bufs=1) as lmx, \
                     tc.tile_pool(name="lmw", bufs=3) as lmwp, \
                     tc.tile_pool(name="lmo", bufs=4) as lmo, \
                     tc.tile_pool(name="pslm", bufs=6, space="PSUM") as pslm:
                    NR = TALL // TM          # 8 token chunks
                    NRR = TALL // 512        # 4 psum column groups
                    rhs = [[None] * NRR for _ in range(KH)]
                    for kc in range(KH):
                        for rr in range(NRR):
                            t_ = lmx.tile([128, 512], f32r, tag=f"rhs{kc}_{rr}", name=f"rhs{kc}_{rr}")
                            nc.sync.dma_start(
                                t_[:, 0:TM],
                                xg[2 * rr, kc * 128:(kc + 1) * 128, :])
                            nc.sync.dma_start(
                                t_[:, TM:512],
                                xg[2 * rr + 1, kc * 128:(kc + 1) * 128, :])
                            rhs[kc][rr] = t_
                    for mt in range(VSP // 128):
                        wt = lmwp.tile([128, KH * 128], f32r, tag="lmw")
                        nc.sync.dma_start(wt[:], lmw_d.ap()[mt])
                        for rr in range(NRR):
                            ps = pslm.tile([128, 512], f32, tag="lm")
                            for kc in range(KH):
                                nc.tensor.matmul(
                                    ps[:], wt[:, kc * 128:(kc + 1) * 128],
                                    rhs[kc][rr][:],
                                    start=(kc == 0), stop=(kc == KH - 1))
                            ob = lmo.tile([128, 512], f32, tag="ob")
                            nc.any.tensor_copy(ob[:], ps[:])
                            nc.sync.dma_start(
                                out_d.ap()[mt * 128:(mt + 1) * 128,
                                           rr * 512:(rr + 1) * 512],
                                ob[:])

    nc.compile()
    return nc


def _prep_inputs(inputs, ffn_bf16, ln_scaled):
    import ml_dtypes
    wdtype = ml_dtypes.bfloat16 if ffn_bf16 else np.float32
    ids = np.asarray(inputs["input_ids"], np.int32)          # [B, S]
    retw = np.stack([_swz(np.asarray(inputs["ret_W"][l], np.float32))
                     for l in range(L)])                      # [L, MH, 128, KH*128]
    w1 = np.stack([_swz(np.asarray(inputs["ffn_W1"][l], np.float32))
                   for l in range(L)]).astype(wdtype)
    w2 = np.stack([_swz(np.asarray(inputs["ffn_W2"][l], np.float32))
                   for l in range(L)]).astype(wdtype)
    retb = np.stack([_cols(np.asarray(inputs["ret_b"][l], np.float32), MH)
                     for l in range(L)])
    b1 = np.stack([_cols(np.asarray(inputs["ffn_b1"][l], np.float32), MF)
                   for l in range(L)])
    b2 = np.stack([_cols(np.asarray(inputs["ffn_b2"][l], np.float32), MH)
                   for l in range(L)])
    lmw_full = np.asarray(inputs["lm_W"], np.float32)         # [H, V]
    pos_emb = np.asarray(inputs["pos_emb"], np.float32)       # [S, H]
    wemb = np.ascontiguousarray(np.asarray(inputs["word_emb"], np.float32))

    common = {
        "wemb": wemb, "retw": retw, "retb": retb,
        "w1": w1, "b1": b1, "w2": w2, "b2": b2,
    }
    if ln_scaled:
        slots = [( np.asarray(inputs["emb_ln_s"], np.float32),
                   np.asarray(inputs["emb_ln_b"], np.float32))]
        for l in range(L):
            slots.append((np.asarray(inputs["ln1_s"][l], np.float32),
                          np.asarray(inputs["ln1_b"][l], np.float32)))
            slots.append((np.asarray(inputs["ln2_s"][l], np.float32),
                          np.asarray(inputs["ln2_b"][l], np.float32)))
        slots.append((np.asarray(inputs["fin_ln_s"], np.float32),
                      np.asarray(inputs["fin_ln_b"], np.float32)))
        lns = np.stack([np.stack([_cols(s, MH), _cols(b, MH)]) for s, b in slots])
        common["lns"] = lns

    in_maps = []
    for c in range(NCORES):
        b = c // (NCORES // B)
        s0 = TM * (c % (NCORES // B))
        if s0 == 0:
            hids = ids[b, 0:HALO]
            hpos = np.arange(HALO)
        else:
            hids = ids[b, s0 - HALO:s0]
            hpos = np.arange(s0 - HALO, s0)
        cids = np.concatenate([hids, ids[b, s0:s0 + TM],
                               np.zeros(TPAD - T, np.int32)]).astype(np.int32)
        cpos = np.concatenate([hpos, np.arange(s0, s0 + TM),
                               np.zeros(TPAD - T, np.int64)])
        pos = pos_emb[cpos].reshape(3, 128, H)
        lmw_c = np.zeros((H, VSP), np.float32)
        lmw_c[:, :VS] = lmw_full[:, c * VS:(c + 1) * VS]
        m = dict(common)
        m["mask"] = np.full((128, 1), 0.0 if s0 == 0 else 1.0, np.float32)
        m["ids"] = cids.reshape(3, 128)
        m["pos"] = np.ascontiguousarray(pos)
        m["lmw"] = _swz(lmw_c)
        in_maps.append(m)
    return in_maps


def kernel(**inputs):
    ffn_bf16 = False
    trivial = all(
        np.allclose(np.asarray(inputs[k]), 1.0) for k in
        ("emb_ln_s", "ln1_s", "ln2_s", "fin_ln_s")
    ) and all(
        np.allclose(np.asarray(inputs[k]), 0.0) for k in
        ("emb_ln_b", "ln1_b", "ln2_b", "fin_ln_b")
    )
    ln_scaled = not trivial

    key = (ffn_bf16, ln_scaled)
    if key not in _compiled:
        _compiled[key] = _build(ffn_bf16, ln_scaled)
    nc = _compiled[key]

    in_maps = _prep_inputs(inputs, ffn_bf16, ln_scaled)
    trace = bool(_os.environ.get("KERNEL_TRACE"))
    if trace:
        _maybe_install_trace_hook()
    res = bass_utils.run_bass_kernel_spmd(
        nc, in_maps, core_ids=list(range(NCORES)), trace=trace)
    global LAST_EXEC_NS
    LAST_EXEC_NS = res.exec_time_ns

    logits = np.empty((TALL, V), np.float32)
    for c in range(NCORES):
        logits[:, c * VS:(c + 1) * VS] = res.results[c]["logits"][:VS, :].T
    return logits.reshape(B, S, V)


# revision 8
# speedup vs baseline: 1.2316x; 1.2316x over previous
"""Trainium2 Bass kernel for nn_CRAMForCausalLM.

Sharding: 8-way data-parallel over tokens (each core owns 256 contiguous
tokens of one batch element, plus a 32-token halo so the EMA retention scan
is computed locally — contributions older than 32 steps are damped by 0.5^32
< 3e-10, below f32 noise).  The LM head is vocab-sharded (each core computes
4000 logit rows for all 2048 tokens) fed by a single AllGather of the final
hidden states.  Activations live feature-major ([H, tokens]) on chip so every
GEMM chains without transposes; LayerNorm stats use ones-matmuls on the
TensorEngine; the EMA recurrence uses the VectorEngine tensor_tensor_scan.
"""

import numpy as np

import concourse.bass as bass
import concourse.bacc as bacc
import concourse.tile as tile
import concourse.mybir as mybir
import concourse.bass_utils as bass_utils
import os as _os

LAST_EXEC_NS = None


def _maybe_install_trace_hook():
    import contextlib, ctypes, sys, types
    if "antenv.axon_hooks" in sys.modules:
        return
    lib = ctypes.CDLL("/opt/axon/libaxon_pjrt.so")
    if not hasattr(lib, "axon_start_nrt_profile"):
        return
    lib.axon_start_nrt_profile.argtypes = [ctypes.POINTER(ctypes.c_int64), ctypes.c_size_t]
    lib.axon_start_nrt_profile.restype = ctypes.c_int64
    lib.axon_stop_nrt_profile.argtypes = [ctypes.c_char_p]
    lib.axon_stop_nrt_profile.restype = ctypes.c_int64

    @contextlib.contextmanager
    def _hook(output_dir, device_ids):
        import jax
        jax.devices()
        if device_ids:
            ids = (ctypes.c_int64 * len(device_ids))(*device_ids)
            rc = lib.axon_start_nrt_profile(ids, len(device_ids))
        else:
            rc = lib.axon_start_nrt_profile(None, 0)
        if rc != 0:
            raise RuntimeError(f"axon_start_nrt_profile rc={rc}")
        try:
            yield
        finally:
            lib.axon_stop_nrt_profile(str(output_dir).encode())

    mod = types.ModuleType("antenv.axon_hooks")
    mod.get_axon_ntff_profile_hook = lambda: _hook
    mod.set_axon_ntff_profile_hook = lambda h: None
    sys.modules["antenv.axon_hooks"] = mod

AF = mybir.ActivationFunctionType
OP = mybir.AluOpType

B, S, H, F, L, V = 2, 1024, 1024, 4096, 8, 32000
EPS = 1e-5
NCORES = 8
HALO = 32
TM = 256            # main tokens per core
T = TM + HALO       # 288 tokens processed per core
TPAD = 384          # padded to 3 x 128 for the embedding gather
KH = H // 128       # 8 k-chunks over H
MH = H // 128       # 8 m-tiles over H
MF = F // 128       # 32 m-tiles over F
VS = V // NCORES    # 4000 vocab rows per core
VSP = 4096          # padded vocab rows per core
TALL = B * S        # 2048 total tokens

f32 = mybir.dt.float32
f32r = mybir.dt.float32r
bf16 = mybir.dt.bfloat16
i32 = mybir.dt.int32

_compiled = {}


def _swz(w, kp=128, mf=128):
    """[K, M] -> [mt, kp, kc*mf] so lhsT tile (mt, kc) = sbuf[:, kc*mf:(kc+1)*mf]."""
    K, M = w.shape
    kc, mt = K // kp, M // mf
    return np.ascontiguousarray(
        w.reshape(kc, kp, mt, mf).transpose(2, 1, 0, 3).reshape(mt, kp, kc * mf)
    )


def _cols(v, mt, width=128):
    """[M] -> [width, mt] so column j is v[j*width:(j+1)*width]."""
    return np.ascontiguousarray(v.reshape(mt, width).T)


def _build(gemm_bf16, ln_scaled):
    """Build + compile the Bass program.

    gemm_bf16: all large GEMMs (ret/ffn/lm + LN stats) in bf16; the
               residual stream stays f32r.  Else float32r everywhere.
    ln_scaled: apply LayerNorm scale/bias tensors (else they are known
               to be identity and are skipped)
    """
    nc = bacc.Bacc("TRN2", target_bir_lowering=False, debug=False,
                   num_devices=NCORES)
    wdt = bf16 if gemm_bf16 else f32r

    # ---- DRAM I/O ----
    ids_d = nc.dram_tensor("ids", [3, 128], i32, kind="ExternalInput")
    pos_d = nc.dram_tensor("pos", [3, 128, H], f32, kind="ExternalInput")
    wemb_d = nc.dram_tensor("wemb", [V, H], f32, kind="ExternalInput")
    retw_d = nc.dram_tensor("retw", [L, MH, 128, KH * 128], wdt, kind="ExternalInput")
    retb_d = nc.dram_tensor("retb", [L, 128, MH], f32, kind="ExternalInput")
    w1_d = nc.dram_tensor("w1", [L, MF, 128, KH * 128], wdt, kind="ExternalInput")
    b1_d = nc.dram_tensor("b1", [L, 128, MF], f32, kind="ExternalInput")
    w2_d = nc.dram_tensor("w2", [L, MH, 128, MF * 128], wdt, kind="ExternalInput")
    b2_d = nc.dram_tensor("b2", [L, 128, MH], f32, kind="ExternalInput")
    lmw_d = nc.dram_tensor("lmw", [VSP // 128, 128, KH * 128], wdt, kind="ExternalInput")
    mask_d = nc.dram_tensor("mask", [128, 1], f32, kind="ExternalInput")
    if ln_scaled:
        # 2 (s, b) x (emb, per-layer ln1, per-layer ln2, fin): [128, MH] each
        lns_d = nc.dram_tensor("lns", [2 * L + 2, 2, 128, MH], f32, kind="ExternalInput")
    out_d = nc.dram_tensor("logits", [VSP, TALL], f32, kind="ExternalOutput")

    with tile.TileContext(nc) as tc:
        with tc.tile_pool(name="per", bufs=1) as per, \
             tc.tile_pool(name="gpool", bufs=1) as gpool:
            # persistent tiles
            xt = [per.tile([128, T], f32r, tag=f"xt{k}", name=f"xt{k}") for k in range(KH)]
            if gemm_bf16:
                xtb = [per.tile([128, T], bf16, tag=f"xtb{k}", name=f"xtb{k}")
                       for k in range(KH)]
                hb = [per.tile([128, T], bf16, tag=f"hb{k}", name=f"hb{k}")
                      for k in range(KH)]
            else:
                xtb = hb = None
            y1 = [per.tile([128, T], f32r, tag=f"y1{k}", name=f"y1{k}") for k in range(KH)]
            hres = [per.tile([128, T], f32r, tag=f"h{k}", name=f"h{k}") for k in range(KH)]
            g = [gpool.tile([128, T], wdt, tag=f"g{k}", name=f"g{k}") for k in range(MF)]
            half = per.tile([128, T], f32)
            nc.gpsimd.memset(half[:], 0.5)
            ones_f = per.tile([128, 1], f32)
            nc.gpsimd.memset(ones_f[:], 1.0)
            ones = per.tile([128, 1], bf16 if gemm_bf16 else f32r)
            nc.vector.tensor_copy(ones[:], ones_f[:])
            onesr_f = per.tile([1, 128], f32)
            nc.gpsimd.memset(onesr_f[:], 1.0)
            onesr = per.tile([1, 128], f32r)
            nc.vector.tensor_copy(onesr[:], onesr_f[:])
            mask = per.tile([128, 1], f32)
            nc.sync.dma_start(mask[:], mask_d.ap())
            epsc = per.tile([128, 1], f32)
            nc.gpsimd.memset(epsc[:], EPS)
            ident = per.tile([128, 128], f32)
            from concourse.masks import make_identity
            make_identity(nc, ident[:])
            if ln_scaled:
                lnt = per.tile([128, (2 * L + 2) * 2 * MH], f32)
                nc.sync.dma_start(
                    lnt[:],
                    lns_d.ap().rearrange("a b p m -> p (a b m)"))
            else:
                lnt = None

            def ln_cols(slot):
                # returns (scale_cols, bias_cols): APs [128, MH] or None
                if lnt is None:
                    return None, None
                off = slot * 2 * MH
                return lnt[:, off:off + MH], lnt[:, off + MH:off + 2 * MH]

            # ---------- LayerNorm helper (feature-major) ----------
            def layer_norm(ps_stat, ps_bc, tmp, yin, yout, slot, yout2=None):
                """yin: 8 tiles [128, T] f32r; yout/yout2: 8 tiles (any dtype).
                Normalizes over the partition(H) axis per token column."""
                scol, bcol = ln_cols(slot)
                sdt = bf16 if gemm_bf16 else f32r
                if gemm_bf16:
                    yb = []
                    for k in range(KH):
                        t_ = tmp.tile([128, T], bf16, tag="ybf", name="ybf")
                        nc.vector.tensor_copy(t_[:], yin[k][:].bitcast(f32))
                        yb.append(t_)
                else:
                    yb = yin
                sq = []
                for k in range(KH):
                    s = tmp.tile([128, T], sdt, tag="sq", name="sq")
                    nc.vector.tensor_tensor(s[:], yb[k][:], yb[k][:], OP.mult)
                    sq.append(s)
                p_sy = ps_stat.tile([1, T], f32, tag="psy")
                p_sq = ps_stat.tile([1, T], f32, tag="psq")
                for k in range(KH):
                    nc.tensor.matmul(p_sy[:], ones[:], yb[k][:],
                                     start=(k == 0), stop=(k == KH - 1))
                for k in range(KH):
                    nc.tensor.matmul(p_sq[:], ones[:], sq[k][:],
                                     start=(k == 0), stop=(k == KH - 1))
                nm = tmp.tile([1, T], f32r, tag="nm")
                nc.vector.tensor_scalar_mul(nm[:], p_sy[:], -1.0 / H)
                v1 = tmp.tile([1, T], f32, tag="v1")
                nc.vector.tensor_scalar_mul(v1[:], p_sq[:], 1.0 / H)
                m2 = tmp.tile([1, T], f32, tag="m2")
                nc.vector.tensor_tensor(m2[:], nm[:].bitcast(f32),
                                        nm[:].bitcast(f32), OP.mult)
                var = tmp.tile([1, T], f32r, tag="var")
                nc.vector.tensor_tensor(var[:], v1[:], m2[:], OP.subtract)
                p_nmb = ps_bc.tile([128, T], f32, tag="pnmb")
                nc.tensor.matmul(p_nmb[:], onesr[:], nm[:], start=True, stop=True)
                p_vb = ps_bc.tile([128, T], f32, tag="pvb")
                nc.tensor.matmul(p_vb[:], onesr[:], var[:], start=True, stop=True)
                sd_b = tmp.tile([128, T], f32, tag="sdb")
                nc.scalar.activation(sd_b[:], p_vb[:], AF.Sqrt, bias=epsc[:])
                r_b = tmp.tile([128, T], f32, tag="rb")
                nc.vector.reciprocal(r_b[:], sd_b[:])
                for k in range(KH):
                    z = tmp.tile([128, T], f32, tag="z", name="z")
                    nc.vector.tensor_tensor(z[:], yin[k][:].bitcast(f32),
                                            p_nmb[:], OP.add)
                    if scol is None:
                        nc.vector.tensor_tensor(yout[k][:], z[:], r_b[:], OP.mult)
                    else:
                        z2 = tmp.tile([128, T], f32, tag="z2", name="z2")
                        nc.vector.tensor_tensor(z2[:], z[:], r_b[:], OP.mult)
                        nc.vector.tensor_scalar(
                            yout[k][:], z2[:],
                            scol[:, k:k + 1], bcol[:, k:k + 1], OP.mult, OP.add)
                    if yout2 is not None:
                        nc.vector.tensor_copy(yout2[k][:],
                                              yout[k][:].bitcast(f32))

            # ================= Embedding =================
            with tc.tile_pool(name="emb", bufs=2) as ep, \
                 tc.tile_pool(name="pse", bufs=3, space="PSUM") as pse:
                for c in range(3):
                    idx = ep.tile([128, 1], i32, tag="idx")
                    nc.sync.dma_start(idx[:], ids_d.ap()[c][:, None])
                    gt = ep.tile([128, H], f32, tag="gt")
                    nc.gpsimd.indirect_dma_start(
                        out=gt[:], out_offset=None, in_=wemb_d.ap(),
                        in_offset=bass.IndirectOffsetOnAxis(ap=idx[:, :1], axis=0))
                    pt = ep.tile([128, H], f32, tag="pt")
                    nc.sync.dma_start(pt[:], pos_d.ap()[c])
                    nc.vector.tensor_tensor(gt[:], gt[:], pt[:], OP.add)
                    cnt = T - 256 if c == 2 else 128
                    for k in range(KH):
                        ptr = pse.tile([128, 128], f32, tag="ptr")
                        nc.tensor.transpose(ptr[:], gt[:, k * 128:(k + 1) * 128],
                                            ident[:])
                        nc.vector.tensor_copy(
                            y1[k][:, c * 128:c * 128 + cnt], ptr[:, :cnt])

            # ================= Layers =================
            with tc.tile_pool(name="wret", bufs=2) as wret, \
                 tc.tile_pool(name="w1p", bufs=3) as w1p, \
                 tc.tile_pool(name="w2p", bufs=2) as w2p, \
                 tc.tile_pool(name="bias", bufs=2) as biasp, \
                 tc.tile_pool(name="tmp", bufs=3) as tmp, \
                 tc.tile_pool(name="psmm", bufs=3, space="PSUM") as psmm, \
                 tc.tile_pool(name="psst", bufs=1, space="PSUM") as ps_stat, \
                 tc.tile_pool(name="psbc", bufs=1, space="PSUM") as ps_bc:

                # embedding LN:  y1 -> xt
                layer_norm(ps_stat, ps_bc, tmp, y1, xt, 0, yout2=xtb)

                for l in range(L):
                    retb = biasp.tile([128, MH], f32, tag="retb")
                    nc.sync.dma_start(retb[:], retb_d.ap()[l])
                    b1 = biasp.tile([128, MF], f32, tag="b1")
                    nc.sync.dma_start(b1[:], b1_d.ap()[l])
                    b2 = biasp.tile([128, MH], f32, tag="b2")
                    nc.sync.dma_start(b2[:], b2_d.ap()[l])

                    # --- retention GEMM + sigmoid + EMA scan ---
                    st = []
                    for mt in range(MH):
                        wt = wret.tile([128, KH * 128], wdt, tag="wret")
                        nc.sync.dma_start(wt[:], retw_d.ap()[l, mt])
                        ps = psmm.tile([128, T], f32, tag="mm")
                        xrhs = xtb if gemm_bf16 else xt
                        for kc in range(KH):
                            nc.tensor.matmul(
                                ps[:], wt[:, kc * 128:(kc + 1) * 128], xrhs[kc][:],
                                start=(kc == 0), stop=(kc == KH - 1))
                        s = tmp.tile([128, T], f32, tag="sig")
                        nc.scalar.activation(s[:], ps[:], AF.Sigmoid,
                                             bias=retb[:, mt:mt + 1])
                        nc.vector.tensor_scalar_mul(
                            s[:, :HALO], s[:, :HALO], mask[:, :1])
                        stt = tmp.tile([128, T], f32, tag="scan")
                        nc.vector.tensor_tensor_scan(
                            stt[:], half[:], s[:], 0.0, OP.mult, OP.add)
                        st.append(stt)
                        # y1 = x + 0.5*scan_state   (f32r rounded on write)
                        nc.vector.scalar_tensor_tensor(
                            y1[mt][:], stt[:], 0.5, xt[mt][:].bitcast(f32),
                            OP.mult, OP.add)

                    # --- LN1: y1 -> hres (f32) ---
                    layer_norm(ps_stat, ps_bc, tmp, y1, hres, 1 + 2 * l, yout2=hb)

                    hg = hb if gemm_bf16 else hres

                    # --- FFN1 + gelu ---
                    for mt in range(MF):
                        wt = w1p.tile([128, KH * 128], wdt, tag="w1")
                        nc.sync.dma_start(wt[:], w1_d.ap()[l, mt])
                        ps = psmm.tile([128, T], f32, tag="mm")
                        for kc in range(KH):
                            nc.tensor.matmul(
                                ps[:], wt[:, kc * 128:(kc + 1) * 128], hg[kc][:],
                                start=(kc == 0), stop=(kc == KH - 1))
                        nc.scalar.activation(g[mt][:], ps[:], AF.Gelu_apprx_tanh,
                                             bias=b1[:, mt:mt + 1])

                    # --- FFN2 ---
                    for mt in range(MH):
                        wt = w2p.tile([128, MF * 128], wdt, tag="w2")
                        nc.sync.dma_start(wt[:], w2_d.ap()[l, mt])
                        ps = psmm.tile([128, T], f32, tag="mm")
                        for kc in range(MF):
                            nc.tensor.matmul(
                                ps[:], wt[:, kc * 128:(kc + 1) * 128], g[kc][:],
                                start=(kc == 0), stop=(kc == MF - 1))
                        # y1 = (ffn + b2) + h    (becomes LN2 input)
                        nc.vector.scalar_tensor_tensor(
                            y1[mt][:], ps[:], b2[:, mt:mt + 1],
                            hres[mt][:].bitcast(f32), OP.add, OP.add)

                    # --- LN2: y1 -> xt ---
                    layer_norm(ps_stat, ps_bc, tmp, y1, xt, 2 + 2 * l, yout2=xtb)

                # final LN: xt -> xf (feeds only the LM head)
                xf = xtb if gemm_bf16 else y1
                layer_norm(ps_stat, ps_bc, tmp, xt, xf, 2 * L + 1)

            # ================= AllGather of final hidden =================
            with tc.tile_pool(name="dram", bufs=1, space="DRAM") as dramp:
                xdt = bf16 if gemm_bf16 else f32r
                bnc = dramp.tile([H, TM], xdt)
                for k in range(KH):
                    nc.sync.dma_start(bnc[k * 128:(k + 1) * 128, :],
                                      xf[k][:, HALO:T])
                xg = dramp.tile([NCORES, H, TM], xdt, addr_space="Shared")
                nc.gpsimd.collective_compute(
                    "AllGather", OP.bypass,
                    replica_groups=[list(range(NCORES))],
                    ins=[bnc.opt()], outs=[xg.opt()])

                # ================= LM head =================
                with tc.tile_pool(name="lmx", bufs=1) as lmx, \
                     tc.tile_pool(name="lmw", bufs=3) as lmwp, \
                     tc.tile_pool(name="lmo", bufs=4) as lmo, \
                     tc.tile_pool(name="pslm", bufs=6, space="PSUM") as pslm:
                    NR = TALL // TM          # 8 token chunks
                    NRR = TALL // 512        # 4 psum column groups
                    rhs = [[None] * NRR for _ in range(KH)]
                    for kc in range(KH):
                        for rr in range(NRR):
                            t_ = lmx.tile([128, 512], xdt, tag=f"rhs{kc}_{rr}", name=f"rhs{kc}_{rr}")
                            nc.sync.dma_start(
                                t_[:, 0:TM],
                                xg[2 * rr, kc * 128:(kc + 1) * 128, :])
                            nc.sync.dma_start(
                                t_[:, TM:512],
                                xg[2 * rr + 1, kc * 128:(kc + 1) * 128, :])
                            rhs[kc][rr] = t_
                    for mt in range(VSP // 128):
                        wt = lmwp.tile([128, KH * 128], wdt, tag="lmw")
                        nc.sync.dma_start(wt[:], lmw_d.ap()[mt])
                        for rr in range(NRR):
                            ps = pslm.tile([128, 512], f32, tag="lm")
                            for kc in range(KH):
                                nc.tensor.matmul(
                                    ps[:], wt[:, kc * 128:(kc + 1) * 128],
                                    rhs[kc][rr][:],
                                    start=(kc == 0), stop=(kc == KH - 1))
                            ob = lmo.tile([128, 512], f32, tag="ob")
                            nc.any.tensor_copy(ob[:], ps[:])
                            nc.sync.dma_start(
                                out_d.ap()[mt * 128:(mt + 1) * 128,
                                           rr * 512:(rr + 1) * 512],
                                ob[:])

    nc.compile()
    return nc


def _prep_inputs(inputs, gemm_bf16, ln_scaled):
    import ml_dtypes
    wdtype = ml_dtypes.bfloat16 if gemm_bf16 else np.float32
    ids = np.asarray(inputs["input_ids"], np.int32)          # [B, S]
    retw = np.stack([_swz(np.asarray(inputs["ret_W"][l], np.float32))
                     for l in range(L)]).astype(wdtype)       # [L, MH, 128, KH*128]
    w1 = np.stack([_swz(np.asarray(inputs["ffn_W1"][l], np.float32))
                   for l in range(L)]).astype(wdtype)
    w2 = np.stack([_swz(np.asarray(inputs["ffn_W2"][l], np.float32))
                   for l in range(L)]).astype(wdtype)
    retb = np.stack([_cols(np.asarray(inputs["ret_b"][l], np.float32), MH)
                     for l in range(L)])
    b1 = np.stack([_cols(np.asarray(inputs["ffn_b1"][l], np.float32), MF)
                   for l in range(L)])
    b2 = np.stack([_cols(np.asarray(inputs["ffn_b2"][l], np.float32), MH)
                   for l in range(L)])
    lmw_full = np.asarray(inputs["lm_W"], np.float32)         # [H, V]
    pos_emb = np.asarray(inputs["pos_emb"], np.float32)       # [S, H]
    wemb = np.ascontiguousarray(np.asarray(inputs["word_emb"], np.float32))

    common = {
        "wemb": wemb, "retw": retw, "retb": retb,
        "w1": w1, "b1": b1, "w2": w2, "b2": b2,
    }
    if ln_scaled:
        slots = [( np.asarray(inputs["emb_ln_s"], np.float32),
                   np.asarray(inputs["emb_ln_b"], np.float32))]
        for l in range(L):
            slots.append((np.asarray(inputs["ln1_s"][l], np.float32),
                          np.asarray(inputs["ln1_b"][l], np.float32)))
            slots.append((np.asarray(inputs["ln2_s"][l], np.float32),
                          np.asarray(inputs["ln2_b"][l], np.float32)))
        slots.append((np.asarray(inputs["fin_ln_s"], np.float32),
                      np.asarray(inputs["fin_ln_b"], np.float32)))
        lns = np.stack([np.stack([_cols(s, MH), _cols(b, MH)]) for s, b in slots])
        common["lns"] = lns

    in_maps = []
    for c in range(NCORES):
        b = c // (NCORES // B)
        s0 = TM * (c % (NCORES // B))
        if s0 == 0:
            hids = ids[b, 0:HALO]
            hpos = np.arange(HALO)
        else:
            hids = ids[b, s0 - HALO:s0]
            hpos = np.arange(s0 - HALO, s0)
        cids = np.concatenate([hids, ids[b, s0:s0 + TM],
                               np.zeros(TPAD - T, np.int32)]).astype(np.int32)
        cpos = np.concatenate([hpos, np.arange(s0, s0 + TM),
                               np.zeros(TPAD - T, np.int64)])
        pos = pos_emb[cpos].reshape(3, 128, H)
        lmw_c = np.zeros((H, VSP), np.float32)
        lmw_c[:, :VS] = lmw_full[:, c * VS:(c + 1) * VS]
        m = dict(common)
        m["mask"] = np.full((128, 1), 0.0 if s0 == 0 else 1.0, np.float32)
        m["ids"] = cids.reshape(3, 128)
        m["pos"] = np.ascontiguousarray(pos)
        m["lmw"] = _swz(lmw_c).astype(wdtype)
        in_maps.append(m)
    return in_maps


def kernel(**inputs):
    gemm_bf16 = _os.environ.get("KERNEL_GEMM_DT", "bf16") == "bf16"
    trivial = all(
        np.allclose(np.asarray(inputs[k]), 1.0) for k in
        ("emb_ln_s", "ln1_s", "ln2_s", "fin_ln_s")
    ) and all(
        np.allclose(np.asarray(inputs[k]), 0.0) for k in
        ("emb_ln_b", "ln1_b", "ln2_b", "fin_ln_b")
    )
    ln_scaled = not trivial

    key = (gemm_bf16, ln_scaled)
    if key not in _compiled:
        _compiled[key] = _build(gemm_bf16, ln_scaled)
    nc = _compiled[key]

    in_maps = _prep_inputs(inputs, gemm_bf16, ln_scaled)
    trace = bool(_os.environ.get("KERNEL_TRACE"))
    if trace:
        _maybe_install_trace_hook()
    res = bass_utils.run_bass_kernel_spmd(
        nc, in_maps, core_ids=list(range(NCORES)), trace=trace)
    global LAST_EXEC_NS
    LAST_EXEC_NS = res.exec_time_ns

    logits = np.empty((TALL, V), np.float32)
    for c in range(NCORES):
        logits[:, c * VS:(c + 1) * VS] = res.results[c]["logits"][:VS, :].T
    return logits.reshape(B, S, V)


# revision 9
# speedup vs baseline: 1.3409x; 1.0888x over previous
"""Trainium2 Bass kernel for nn_CRAMForCausalLM.

Sharding: 8-way data-parallel over tokens (each core owns 256 contiguous
tokens of one batch element, plus a 32-token halo so the EMA retention scan
is computed locally — contributions older than 32 steps are damped by 0.5^32
< 3e-10, below f32 noise).  The LM head is vocab-sharded (each core computes
4000 logit rows for all 2048 tokens) fed by a single AllGather of the final
hidden states.  Activations live feature-major ([H, tokens]) on chip so every
GEMM chains without transposes; LayerNorm stats use ones-matmuls on the
TensorEngine; the EMA recurrence uses the VectorEngine tensor_tensor_scan.
"""

import numpy as np

import concourse.bass as bass
import concourse.bacc as bacc
import concourse.tile as tile
import concourse.mybir as mybir
import concourse.bass_utils as bass_utils
import os as _os

LAST_EXEC_NS = None


def _maybe_install_trace_hook():
    import contextlib, ctypes, sys, types
    if "antenv.axon_hooks" in sys.modules:
        return
    lib = ctypes.CDLL("/opt/axon/libaxon_pjrt.so")
    if not hasattr(lib, "axon_start_nrt_profile"):
        return
    lib.axon_start_nrt_profile.argtypes = [ctypes.POINTER(ctypes.c_int64), ctypes.c_size_t]
    lib.axon_start_nrt_profile.restype = ctypes.c_int64
    lib.axon_stop_nrt_profile.argtypes = [ctypes.c_char_p]
    lib.axon_stop_nrt_profile.restype = ctypes.c_int64

    @contextlib.contextmanager
    def _hook(output_dir, device_ids):
        import jax
        jax.devices()
        if device_ids:
            ids = (ctypes.c_int64 * len(device_ids))(*device_ids)
            rc = lib.axon_start_nrt_profile(ids, len(device_ids))
        else:
            rc = lib.axon_start_nrt_profile(None, 0)
        if rc != 0:
            raise RuntimeError(f"axon_start_nrt_profile rc={rc}")
        try:
            yield
        finally:
            lib.axon_stop_nrt_profile(str(output_dir).encode())

    mod = types.ModuleType("antenv.axon_hooks")
    mod.get_axon_ntff_profile_hook = lambda: _hook
    mod.set_axon_ntff_profile_hook = lambda h: None
    sys.modules["antenv.axon_hooks"] = mod

AF = mybir.ActivationFunctionType
OP = mybir.AluOpType

B, S, H, F, L, V = 2, 1024, 1024, 4096, 8, 32000
EPS = 1e-5
NCORES = 8
HALO = 32
TM = 256            # main tokens per core
T = TM + HALO       # 288 tokens processed per core
TPAD = 384          # padded to 3 x 128 for the embedding gather
KH = H // 128       # 8 k-chunks over H
MH = H // 128       # 8 m-tiles over H
MF = F // 128       # 32 m-tiles over F
VS = V // NCORES    # 4000 vocab rows per core
VSP = 4096          # padded vocab rows per core
TALL = B * S        # 2048 total tokens

f32 = mybir.dt.float32
f32r = mybir.dt.float32r
bf16 = mybir.dt.bfloat16
i32 = mybir.dt.int32

_compiled = {}


def _swz(w, kp=128, mf=128):
    """[K, M] -> [mt, kp, kc*mf] so lhsT tile (mt, kc) = sbuf[:, kc*mf:(kc+1)*mf]."""
    K, M = w.shape
    kc, mt = K // kp, M // mf
    return np.ascontiguousarray(
        w.reshape(kc, kp, mt, mf).transpose(2, 1, 0, 3).reshape(mt, kp, kc * mf)
    )


def _cols(v, mt, width=128):
    """[M] -> [width, mt] so column j is v[j*width:(j+1)*width]."""
    return np.ascontiguousarray(v.reshape(mt, width).T)


def _build(gemm_bf16, ln_scaled):
    """Build + compile the Bass program.

    gemm_bf16: all large GEMMs (ret/ffn/lm + LN stats) in bf16; the
               residual stream stays f32r.  Else float32r everywhere.
    ln_scaled: apply LayerNorm scale/bias tensors (else they are known
               to be identity and are skipped)
    """
    nc = bacc.Bacc("TRN2", target_bir_lowering=False, debug=False,
                   num_devices=NCORES)
    wdt = bf16 if gemm_bf16 else f32r

    # ---- DRAM I/O ----
    ids_d = nc.dram_tensor("ids", [3, 128], i32, kind="ExternalInput")
    pos_d = nc.dram_tensor("pos", [3, 128, H], f32, kind="ExternalInput")
    wemb_d = nc.dram_tensor("wemb", [V, H], f32, kind="ExternalInput")
    retw_d = nc.dram_tensor("retw", [L, MH, 128, KH * 128], wdt, kind="ExternalInput")
    retb_d = nc.dram_tensor("retb", [L, 128, MH], f32, kind="ExternalInput")
    w1_d = nc.dram_tensor("w1", [L, MF, 128, KH * 128], wdt, kind="ExternalInput")
    b1_d = nc.dram_tensor("b1", [L, 128, MF], f32, kind="ExternalInput")
    w2_d = nc.dram_tensor("w2", [L, MH, 128, MF * 128], wdt, kind="ExternalInput")
    b2_d = nc.dram_tensor("b2", [L, 128, MH], f32, kind="ExternalInput")
    lmw_d = nc.dram_tensor("lmw", [VSP // 128, 128, KH * 128], wdt, kind="ExternalInput")
    mask_d = nc.dram_tensor("mask", [128, 1], f32, kind="ExternalInput")
    if ln_scaled:
        # 2 (s, b) x (emb, per-layer ln1, per-layer ln2, fin): [128, MH] each
        lns_d = nc.dram_tensor("lns", [2 * L + 2, 2, 128, MH], f32, kind="ExternalInput")
    out_d = nc.dram_tensor("logits", [VSP, TALL], f32, kind="ExternalOutput")

    with tile.TileContext(nc) as tc:
        with tc.tile_pool(name="per", bufs=1) as per, \
             tc.tile_pool(name="gpool", bufs=1) as gpool:
            # persistent tiles
            xt = [per.tile([128, T], f32r, tag=f"xt{k}", name=f"xt{k}") for k in range(KH)]
            if gemm_bf16:
                xtb = [per.tile([128, T], bf16, tag=f"xtb{k}", name=f"xtb{k}")
                       for k in range(KH)]
                hb = [per.tile([128, T], bf16, tag=f"hb{k}", name=f"hb{k}")
                      for k in range(KH)]
            else:
                xtb = hb = None
            y1 = [per.tile([128, T], f32r, tag=f"y1{k}", name=f"y1{k}") for k in range(KH)]
            hres = [per.tile([128, T], f32r, tag=f"h{k}", name=f"h{k}") for k in range(KH)]
            g = [gpool.tile([128, T], wdt, tag=f"g{k}", name=f"g{k}") for k in range(MF)]
            half = per.tile([128, T], f32)
            nc.gpsimd.memset(half[:], 0.5)
            ones_f = per.tile([128, 1], f32)
            nc.gpsimd.memset(ones_f[:], 1.0)
            ones = per.tile([128, 1], bf16 if gemm_bf16 else f32r)
            nc.vector.tensor_copy(ones[:], ones_f[:])
            onesr_f = per.tile([1, 128], f32)
            nc.gpsimd.memset(onesr_f[:], 1.0)
            onesr = per.tile([1, 128], f32r)
            nc.vector.tensor_copy(onesr[:], onesr_f[:])
            mask = per.tile([128, 1], f32)
            nc.sync.dma_start(mask[:], mask_d.ap())
            epsc = per.tile([128, 1], f32)
            nc.gpsimd.memset(epsc[:], EPS)
            ident = per.tile([128, 128], f32)
            from concourse.masks import make_identity
            make_identity(nc, ident[:])
            if ln_scaled:
                lnt = per.tile([128, (2 * L + 2) * 2 * MH], f32)
                nc.sync.dma_start(
                    lnt[:],
                    lns_d.ap().rearrange("a b p m -> p (a b m)"))
            else:
                lnt = None

            def ln_cols(slot):
                # returns (scale_cols, bias_cols): APs [128, MH] or None
                if lnt is None:
                    return None, None
                off = slot * 2 * MH
                return lnt[:, off:off + MH], lnt[:, off + MH:off + 2 * MH]

            # ---------- LayerNorm helper (feature-major) ----------
            def layer_norm(ps_stat, ps_bc, tmp, yin, yout, slot, yout2=None):
                """yin: 8 tiles [128, T] f32r; yout/yout2: 8 tiles (any dtype).
                Normalizes over the partition(H) axis per token column."""
                scol, bcol = ln_cols(slot)
                sdt = bf16 if gemm_bf16 else f32r
                if gemm_bf16:
                    yb = []
                    for k in range(KH):
                        t_ = tmp.tile([128, T], bf16, tag="ybf", name="ybf")
                        nc.vector.tensor_copy(t_[:], yin[k][:].bitcast(f32))
                        yb.append(t_)
                else:
                    yb = yin
                sq = []
                for k in range(KH):
                    s = tmp.tile([128, T], sdt, tag="sq", name="sq")
                    nc.vector.tensor_tensor(s[:], yb[k][:], yb[k][:], OP.mult)
                    sq.append(s)
                p_st = ps_stat.tile([33, T], f32, tag="pst")
                p_sy = p_st[0:1, :]
                p_sq = p_st[32:33, :]
                for k in range(KH):
                    nc.tensor.matmul(p_sy, ones[:], yb[k][:],
                                     start=(k == 0), stop=(k == KH - 1))
                for k in range(KH):
                    nc.tensor.matmul(p_sq, ones[:], sq[k][:],
                                     start=(k == 0), stop=(k == KH - 1))
                nm = tmp.tile([1, T], f32r, tag="nm")
                nc.vector.tensor_scalar_mul(nm[:], p_sy, -1.0 / H)
                v1 = tmp.tile([1, T], f32, tag="v1")
                nc.vector.tensor_scalar_mul(v1[:], p_sq, 1.0 / H)
                m2 = tmp.tile([1, T], f32, tag="m2")
                nc.vector.tensor_tensor(m2[:], nm[:].bitcast(f32),
                                        nm[:].bitcast(f32), OP.mult)
                var = tmp.tile([1, T], f32r, tag="var")
                nc.vector.tensor_tensor(var[:], v1[:], m2[:], OP.subtract)
                p_vb = ps_bc.tile([128, T], f32, tag="bc")
                nc.tensor.matmul(p_vb[:], onesr[:], var[:], start=True, stop=True)
                sd_b = tmp.tile([128, T], f32, tag="sdb")
                nc.scalar.activation(sd_b[:], p_vb[:], AF.Sqrt, bias=epsc[:])
                r_b = tmp.tile([128, T], f32, tag="rb")
                nc.vector.reciprocal(r_b[:], sd_b[:])
                p_nmb = ps_bc.tile([128, T], f32, tag="bc")
                nc.tensor.matmul(p_nmb[:], onesr[:], nm[:], start=True, stop=True)
                for k in range(KH):
                    z = tmp.tile([128, T], f32, tag="z", name="z")
                    nc.vector.tensor_tensor(z[:], yin[k][:].bitcast(f32),
                                            p_nmb[:], OP.add)
                    if scol is None:
                        nc.vector.tensor_tensor(yout[k][:], z[:], r_b[:], OP.mult)
                    else:
                        z2 = tmp.tile([128, T], f32, tag="z2", name="z2")
                        nc.vector.tensor_tensor(z2[:], z[:], r_b[:], OP.mult)
                        nc.vector.tensor_scalar(
                            yout[k][:], z2[:],
                            scol[:, k:k + 1], bcol[:, k:k + 1], OP.mult, OP.add)
                    if yout2 is not None:
                        nc.vector.tensor_copy(yout2[k][:],
                                              yout[k][:].bitcast(f32))

            # ================= Embedding =================
            with tc.tile_pool(name="emb", bufs=2) as ep, \
                 tc.tile_pool(name="pse", bufs=3, space="PSUM") as pse:
                for c in range(3):
                    idx = ep.tile([128, 1], i32, tag="idx")
                    nc.sync.dma_start(idx[:], ids_d.ap()[c][:, None])
                    gt = ep.tile([128, H], f32, tag="gt")
                    nc.gpsimd.indirect_dma_start(
                        out=gt[:], out_offset=None, in_=wemb_d.ap(),
                        in_offset=bass.IndirectOffsetOnAxis(ap=idx[:, :1], axis=0))
                    pt = ep.tile([128, H], f32, tag="pt")
                    nc.sync.dma_start(pt[:], pos_d.ap()[c])
                    nc.vector.tensor_tensor(gt[:], gt[:], pt[:], OP.add)
                    cnt = T - 256 if c == 2 else 128
                    for k in range(KH):
                        ptr = pse.tile([128, 128], f32, tag="ptr")
                        nc.tensor.transpose(ptr[:], gt[:, k * 128:(k + 1) * 128],
                                            ident[:])
                        nc.vector.tensor_copy(
                            y1[k][:, c * 128:c * 128 + cnt], ptr[:, :cnt])

            # ================= Layers =================
            with tc.tile_pool(name="wret", bufs=3) as wret, \
                 tc.tile_pool(name="w1p", bufs=4) as w1p, \
                 tc.tile_pool(name="w2p", bufs=3) as w2p, \
                 tc.tile_pool(name="bias", bufs=2) as biasp, \
                 tc.tile_pool(name="tmp", bufs=3) as tmp, \
                 tc.tile_pool(name="psmm", bufs=5, space="PSUM") as psmm, \
                 tc.tile_pool(name="psst", bufs=1, space="PSUM") as ps_stat, \
                 tc.tile_pool(name="psbc", bufs=1, space="PSUM") as ps_bc:

                # embedding LN:  y1 -> xt
                layer_norm(ps_stat, ps_bc, tmp, y1, xt, 0, yout2=xtb)

                for l in range(L):
                    retb = biasp.tile([128, MH], f32, tag="retb")
                    nc.sync.dma_start(retb[:], retb_d.ap()[l])
                    b1 = biasp.tile([128, MF], f32, tag="b1")
                    nc.sync.dma_start(b1[:], b1_d.ap()[l])
                    b2 = biasp.tile([128, MH], f32, tag="b2")
                    nc.sync.dma_start(b2[:], b2_d.ap()[l])

                    # --- retention GEMM + sigmoid + EMA scan ---
                    st = []
                    for mt in range(MH):
                        wt = wret.tile([128, KH * 128], wdt, tag="wret")
                        nc.sync.dma_start(wt[:], retw_d.ap()[l, mt])
                        ps = psmm.tile([128, T], f32, tag="mm")
                        xrhs = xtb if gemm_bf16 else xt
                        for kc in range(KH):
                            nc.tensor.matmul(
                                ps[:], wt[:, kc * 128:(kc + 1) * 128], xrhs[kc][:],
                                start=(kc == 0), stop=(kc == KH - 1))
                        s = tmp.tile([128, T], f32, tag="sig")
                        nc.scalar.activation(s[:], ps[:], AF.Sigmoid,
                                             bias=retb[:, mt:mt + 1])
                        nc.vector.tensor_scalar_mul(
                            s[:, :HALO], s[:, :HALO], mask[:, :1])
                        stt = tmp.tile([128, T], f32, tag="scan")
                        nc.vector.tensor_tensor_scan(
                            stt[:], half[:], s[:], 0.0, OP.mult, OP.add)
                        st.append(stt)
                        # y1 = x + 0.5*scan_state   (f32r rounded on write)
                        nc.vector.scalar_tensor_tensor(
                            y1[mt][:], stt[:], 0.5, xt[mt][:].bitcast(f32),
                            OP.mult, OP.add)

                    # --- LN1: y1 -> hres (f32) ---
                    layer_norm(ps_stat, ps_bc, tmp, y1, hres, 1 + 2 * l, yout2=hb)

                    hg = hb if gemm_bf16 else hres

                    # --- FFN1 + gelu ---
                    for mt in range(MF):
                        wt = w1p.tile([128, KH * 128], wdt, tag="w1")
                        nc.sync.dma_start(wt[:], w1_d.ap()[l, mt])
                        ps = psmm.tile([128, T], f32, tag="mm")
                        for kc in range(KH):
                            nc.tensor.matmul(
                                ps[:], wt[:, kc * 128:(kc + 1) * 128], hg[kc][:],
                                start=(kc == 0), stop=(kc == KH - 1))
                        nc.scalar.activation(g[mt][:], ps[:], AF.Gelu_apprx_tanh,
                                             bias=b1[:, mt:mt + 1])

                    # --- FFN2 ---
                    for mt in range(MH):
                        wt = w2p.tile([128, MF * 128], wdt, tag="w2")
                        nc.sync.dma_start(wt[:], w2_d.ap()[l, mt])
                        ps = psmm.tile([128, T], f32, tag="mm")
                        for kc in range(MF):
                            nc.tensor.matmul(
                                ps[:], wt[:, kc * 128:(kc + 1) * 128], g[kc][:],
                                start=(kc == 0), stop=(kc == MF - 1))
                        # y1 = (ffn + b2) + h    (becomes LN2 input)
                        nc.vector.scalar_tensor_tensor(
                            y1[mt][:], ps[:], b2[:, mt:mt + 1],
                            hres[mt][:].bitcast(f32), OP.add, OP.add)

                    # --- LN2: y1 -> xt ---
                    layer_norm(ps_stat, ps_bc, tmp, y1, xt, 2 + 2 * l, yout2=xtb)

                # final LN: xt -> xf (feeds only the LM head)
                xf = xtb if gemm_bf16 else y1
                layer_norm(ps_stat, ps_bc, tmp, xt, xf, 2 * L + 1)

            # ================= AllGather of final hidden =================
            with tc.tile_pool(name="dram", bufs=1, space="DRAM") as dramp:
                xdt = bf16 if gemm_bf16 else f32r
                bnc = dramp.tile([H, TM], xdt)
                for k in range(KH):
                    nc.sync.dma_start(bnc[k * 128:(k + 1) * 128, :],
                                      xf[k][:, HALO:T])
                xg = dramp.tile([NCORES, H, TM], xdt, addr_space="Shared")
                nc.gpsimd.collective_compute(
                    "AllGather", OP.bypass,
                    replica_groups=[list(range(NCORES))],
                    ins=[bnc.opt()], outs=[xg.opt()])

                # ================= LM head =================
                with tc.tile_pool(name="lmx", bufs=1) as lmx, \
                     tc.tile_pool(name="lmw", bufs=3) as lmwp, \
                     tc.tile_pool(name="lmo", bufs=4) as lmo, \
                     tc.tile_pool(name="pslm", bufs=6, space="PSUM") as pslm:
                    NR = TALL // TM          # 8 token chunks
                    NRR = TALL // 512        # 4 psum column groups
                    rhs = [[None] * NRR for _ in range(KH)]
                    for kc in range(KH):
                        for rr in range(NRR):
                            t_ = lmx.tile([128, 512], xdt, tag=f"rhs{kc}_{rr}", name=f"rhs{kc}_{rr}")
                            nc.sync.dma_start(
                                t_[:, 0:TM],
                                xg[2 * rr, kc * 128:(kc + 1) * 128, :])
                            nc.sync.dma_start(
                                t_[:, TM:512],
                                xg[2 * rr + 1, kc * 128:(kc + 1) * 128, :])
                            rhs[kc][rr] = t_
                    for mt in range(VSP // 128):
                        wt = lmwp.tile([128, KH * 128], wdt, tag="lmw")
                        nc.sync.dma_start(wt[:], lmw_d.ap()[mt])
                        for rr in range(NRR):
                            ps = pslm.tile([128, 512], f32, tag="lm")
                            for kc in range(KH):
                                nc.tensor.matmul(
                                    ps[:], wt[:, kc * 128:(kc + 1) * 128],
                                    rhs[kc][rr][:],
                                    start=(kc == 0), stop=(kc == KH - 1))
                            ob = lmo.tile([128, 512], f32, tag="ob")
                            nc.any.tensor_copy(ob[:], ps[:])
                            nc.sync.dma_start(
                                out_d.ap()[mt * 128:(mt + 1) * 128,
                                           rr * 512:(rr + 1) * 512],
                                ob[:])

    nc.compile()
    return nc


def _prep_inputs(inputs, gemm_bf16, ln_scaled):
    import ml_dtypes
    wdtype = ml_dtypes.bfloat16 if gemm_bf16 else np.float32
    ids = np.asarray(inputs["input_ids"], np.int32)          # [B, S]
    retw = np.stack([_swz(np.asarray(inputs["ret_W"][l], np.float32))
                     for l in range(L)]).astype(wdtype)       # [L, MH, 128, KH*128]
    w1 = np.stack([_swz(np.asarray(inputs["ffn_W1"][l], np.float32))
                   for l in range(L)]).astype(wdtype)
    w2 = np.stack([_swz(np.asarray(inputs["ffn_W2"][l], np.float32))
                   for l in range(L)]).astype(wdtype)
    retb = np.stack([_cols(np.asarray(inputs["ret_b"][l], np.float32), MH)
                     for l in range(L)])
    b1 = np.stack([_cols(np.asarray(inputs["ffn_b1"][l], np.float32), MF)
                   for l in range(L)])
    b2 = np.stack([_cols(np.asarray(inputs["ffn_b2"][l], np.float32), MH)
                   for l in range(L)])
    lmw_full = np.asarray(inputs["lm_W"], np.float32)         # [H, V]
    pos_emb = np.asarray(inputs["pos_emb"], np.float32)       # [S, H]
    wemb = np.ascontiguousarray(np.asarray(inputs["word_emb"], np.float32))

    common = {
        "wemb": wemb, "retw": retw, "retb": retb,
        "w1": w1, "b1": b1, "w2": w2, "b2": b2,
    }
    if ln_scaled:
        slots = [( np.asarray(inputs["emb_ln_s"], np.float32),
                   np.asarray(inputs["emb_ln_b"], np.float32))]
        for l in range(L):
            slots.append((np.asarray(inputs["ln1_s"][l], np.float32),
                          np.asarray(inputs["ln1_b"][l], np.float32)))
            slots.append((np.asarray(inputs["ln2_s"][l], np.float32),
                          np.asarray(inputs["ln2_b"][l], np.float32)))
        slots.append((np.asarray(inputs["fin_ln_s"], np.float32),
                      np.asarray(inputs["fin_ln_b"], np.float32)))
        lns = np.stack([np.stack([_cols(s, MH), _cols(b, MH)]) for s, b in slots])
        common["lns"] = lns

    in_maps = []
    for c in range(NCORES):
        b = c // (NCORES // B)
        s0 = TM * (c % (NCORES // B))
        if s0 == 0:
            hids = ids[b, 0:HALO]
            hpos = np.arange(HALO)
        else:
            hids = ids[b, s0 - HALO:s0]
            hpos = np.arange(s0 - HALO, s0)
        cids = np.concatenate([hids, ids[b, s0:s0 + TM],
                               np.zeros(TPAD - T, np.int32)]).astype(np.int32)
        cpos = np.concatenate([hpos, np.arange(s0, s0 + TM),
                               np.zeros(TPAD - T, np.int64)])
        pos = pos_emb[cpos].reshape(3, 128, H)
        lmw_c = np.zeros((H, VSP), np.float32)
        lmw_c[:, :VS] = lmw_full[:, c * VS:(c + 1) * VS]
        m = dict(common)
        m["mask"] = np.full((128, 1), 0.0 if s0 == 0 else 1.0, np.float32)
        m["ids"] = cids.reshape(3, 128)
        m["pos"] = np.ascontiguousarray(pos)
        m["lmw"] = _swz(lmw_c).astype(wdtype)
        in_maps.append(m)
    return in_maps


def kernel(**inputs):
    gemm_bf16 = _os.environ.get("KERNEL_GEMM_DT", "bf16") == "bf16"
    trivial = all(
        np.allclose(np.asarray(inputs[k]), 1.0) for k in
        ("emb_ln_s", "ln1_s", "ln2_s", "fin_ln_s")
    ) and all(
        np.allclose(np.asarray(inputs[k]), 0.0) for k in
        ("emb_ln_b", "ln1_b", "ln2_b", "fin_ln_b")
    )
    ln_scaled = not trivial

    key = (gemm_bf16, ln_scaled)
    if key not in _compiled:
        _compiled[key] = _build(gemm_bf16, ln_scaled)
    nc = _compiled[key]

    in_maps = _prep_inputs(inputs, gemm_bf16, ln_scaled)
    trace = bool(_os.environ.get("KERNEL_TRACE"))
    if trace:
        _maybe_install_trace_hook()
    res = bass_utils.run_bass_kernel_spmd(
        nc, in_maps, core_ids=list(range(NCORES)), trace=trace)
    global LAST_EXEC_NS
    LAST_EXEC_NS = res.exec_time_ns

    logits = np.empty((TALL, V), np.float32)
    for c in range(NCORES):
        logits[:, c * VS:(c + 1) * VS] = res.results[c]["logits"][:VS, :].T
    return logits.reshape(B, S, V)
